# revision 1
# baseline (speedup 1.0000x reference)
"""Trainium2 Bass kernel for nn_BKCoreHyperbolicIntegration (8 NeuronCores).

Reference computation:
    he_diag[b,s] = mean_e( x[b,s,:] @ Wd[e,:] + bd[e] )   # == x @ colmean(Wd) + mean(bd)
    G = 1 / (he_diag - (0 + 0.1j) + 1e-6)                 # complex64
    gate = sigmoid(gW[0,0]*Re(G) + gW[0,1]*Im(G) + gb[0]) # [B,S]
    gated = attention_weights * gate[:, None, :, None]
    out = gated / (gated.sum(-1, keepdims=True) + 1e-6)

Algebra used:
  * mean_e(x @ Wd.T + bd) == x @ colmean(Wd) + mean(bd): the [D,D] projection
    collapses to a matvec against the column mean of Wd (verified 5.6e-7
    max rel err vs the reference).
  * h0_super / h0_sub in the reference are dead code (deleted) -> skipped.
  * With z = 0.1j and d := he + EPS:  Re G = d/(d^2+0.01), Im G = 0.1/(d^2+0.01).

Sharding: the S (row) axis of attention_weights is split across the 8 cores
(core k owns rows [128k, 128k+128) for every b,h).  Each core computes
gate[b, s_chunk] on-device from its x row-slice.  The Wd column-sum is
computed on-device: each core PE-reduces its own 256-row slice of Wd and the
partial sums are combined with an 8-core AllReduce (COLLECTIVE_MODE=True;
set False to fold colsum(Wd) on the host instead).

Raw-Block implementation.  Toolchain behaviors discovered empirically (this
compiler/runtime rejects or miscompiles several paths):
  * TileContext's auto-generated sync exceeds the compiler's per-instruction
    sync-wait limit ("Too many sync wait commands") -> all semaphores are
    explicit, kept to 1-2 waits per instruction.
  * InstReciprocal returns inf on HW; InstTensorTensorReduce and custom-DVE
    ops fail codegen -> reciprocal is exp(-ln(x)) on the scalar engine
    (~5e-5 rel err, well inside tolerance).
  * Engines pipeline without RAW interlocks: an op reading data written by
    the SAME engine shortly before sees stale values (worst through the
    scalar-operand port: tensor_scalar scalar1/scalar2 APs, activation
    scale/bias APs).  Every same-engine dependent pair is completion-synced
    via a chain semaphore, and every scalar-port operand is produced by a
    different engine behind a semaphore.
  * DMA completion semaphore quanta are shape-dependent ([1,D] DMAs post 32,
    [128,*] post 16; verified from CoreSim final semaphore values), and
    concurrent DMAs on one semaphore interleave engine-level increments ->
    one-DMA-in-flight-per-semaphore (per ring slot), with tiny header loads
    covered by queue-FIFO ordering (a later DMA's full completion implies
    earlier same-queue DMAs landed).

Engine roles:
  SP     streams attention tiles in (6-slot ring, in-place gating)
  DVE    row-sum reduces, all four multiplies per tile, gate linear algebra
  ACT    denominators via Copy(scale=gate), ln/exp reciprocals, sigmoid,
         output DMAs
  PE     Wd column-sum matmuls (ones.T @ Wd_rows, PSUM-accumulated)
  GPSIMD AllReduce + stride-0 broadcast DMAs
"""

from contextlib import ExitStack

import numpy as np

import concourse.bass as bass
from concourse import mybir
from concourse.bass_utils import run_bass_kernel_spmd

COLLECTIVE_MODE = True
TRACE = False
LAST_EXEC_NS = None
LAST_RESULTS = None

F32 = mybir.dt.float32
AX = mybir.AxisListType
ALU = mybir.AluOpType
ACT_F = mybir.ActivationFunctionType

B, S, H, D = 2, 1024, 16, 2048
N_CORES = 8
S_CHUNK = S // N_CORES
BH = B * H
GROUP = 4
NG = BH // GROUP
RING = 8
EPS = 1e-6
INV_D = 1.0 / D
Q_IN = 16
Q_OUT = 16
Q_CC = 32        # cc_in [1,D] colsum -> DRAM
Q_WS = 32        # cc_out/wsum [1,D] -> SBUF
Q_EX = 16
Q_EXB = 16
THROTTLE = 2     # max in-flight tin transfers ahead
PAUSE_K = 99     # collective: tin index at which SP waits for wbar bcast (off)
CCI_WAIT = False # collective: hold tin stream until AllReduce payload sent
N_HOIST = 5      # collective: reduces hoisted before the gate chain



def build_kernel(use_collective: bool, debug: bool = False,
                 detect_races: bool = True):
    nc = bass.Bass(detect_race_conditions=detect_races)
    attn_in = nc.declare_dram_parameter("attn", [BH, S_CHUNK, S], F32, isOutput=False)
    xs_in = nc.declare_dram_parameter("xs", [B, S_CHUNK, D], F32, isOutput=False)
    if use_collective:
        wd_in = nc.declare_dram_parameter("wd", [D // N_CORES, D], F32, isOutput=False)
    else:
        wsum_in = nc.declare_dram_parameter("wsum", [1, D], F32, isOutput=False)
    bd_in = nc.declare_dram_parameter("bd", [1, D], F32, isOutput=False)
    gwb_in = nc.declare_dram_parameter("gwb", [1, 3], F32, isOutput=False)
    out_d = nc.declare_dram_parameter("out", [BH, S_CHUNK, S], F32, isOutput=True)
    if use_collective:
        cc_in = nc.dram_tensor("cc_in", [1, D], F32)
        cc_out = nc.dram_tensor("cc_out", [1, D], F32, addr_space="Shared")
    extras_dram = nc.dram_tensor("extras_dram", [1, 4], F32)
    if debug:
        dbg_out = nc.declare_dram_parameter("dbg", [128, 64], F32, isOutput=True)

    ctx = ExitStack()
    with ctx:
        sb = lambda shape, name: ctx.enter_context(
            nc.sbuf_tensor(name, shape, F32))
        sem = lambda name: ctx.enter_context(nc.semaphore(name))

        tin = [sb([128, GROUP * S], f"tin{i}") for i in range(RING)]
        rs_all = sb([128, BH], "rs_all")
        den_all = sb([128, BH], "den_all")
        rec_all = sb([128, BH], "rec_all")
        rec_scr = sb([128, BH], "rec_scr")
        sc_all = sb([128, BH], "sc_all")
        xt = [sb([128, D], f"xt{b}") for b in range(B)]
        wsum_sb = sb([1, D], "wsum_sb")
        bd_sb = sb([1, D], "bd_sb")
        dinit = sb([1, 1], "dinit")
        gwb_sb = sb([1, 3], "gwb_sb")
        staging = sb([1, 4], "staging")
        extras_sb = sb([128, 4], "extras_sb")
        gate_sb = sb([128, B], "gate_sb")
        gate_d = sb([128, B], "gate_d")
        ghraw = sb([128, B], "ghraw")
        dcol = sb([128, B], "dcol")
        gden = sb([128, B], "gden")
        grec = sb([128, B], "grec")
        grscr = sb([128, B], "grscr")
        gt1 = sb([128, B], "gt1")
        gt1g = sb([128, B], "gt1g")
        gt2g = sb([128, B], "gt2g")
        glin = sb([128, B], "glin")
        wbar_sb = sb([128, D], "wbar_sb")
        if use_collective:
            wd_t = [sb([128, D], f"wd{i}") for i in range(2)]
            colsum_sb = sb([1, D], "colsum_sb")
            colsum_ps = ctx.enter_context(
                nc.psum_tensor("colsum_ps", [1, D], F32))
            ones_col = sb([128, 1], "ones_col")

        s_in_slot = [sem(f"s_in{j}") for j in range(RING)]
        s_out_slot = [sem(f"s_out{j}") for j in range(RING)]
        s_x = [sem(f"s_x{b}") for b in range(B)]
        if use_collective:
            s_wd = [sem(f"s_wd{i}") for i in range(2)]
        s_cci = sem("s_cci")
        s_ws = sem("s_ws")
        s_exo = sem("s_exo")
        s_exb = sem("s_exb")
        s_ones = sem("s_ones")
        s_pe = sem("s_pe")        # PE colsum done
        s_peb = sem("s_peb")      # PE wbar broadcast done
        s_colsum_sb = sem("s_colsum_sb")
        s_cc = sem("s_cc")
        s_dinit = sem("s_dinit")
        s_staging = sem("s_staging")
        s_gden = sem("s_gden")
        s_grec = sem("s_grec")
        s_lin = sem("s_lin")
        s_gate = sem("s_gate")
        s_gated = sem("s_gated")
        s_rs = sem("s_rs")
        s_sc = sem("s_sc")
        s_mul_dve = sem("s_mul_dve")
        s_sink = sem("s_sink")
        s_vchain = sem("s_vchain")
        s_achain = sem("s_achain")
        if debug:
            dbg = sb([128, 64], "dbg_sb")
            s_dbg = sem("s_dbg")

        with nc.Block() as block:

            @block.sync
            def _(sync):
                # smalls first (no direct waiters: covered via queue FIFO by
                # the first tracked DMA's full completion)
                sync.dma_start(bd_sb[:], bd_in[:]).then_inc(s_sink, 16)
                sync.dma_start(gwb_sb[:], gwb_in[:]).then_inc(s_sink, 16)
                if use_collective:
                    for i in range(2):
                        sync.dma_start(
                            wd_t[i][:], wd_in[i * 128:(i + 1) * 128, :]
                        ).then_inc(s_wd[i], 16)
                else:
                    sync.dma_start(wsum_sb[:], wsum_in[:]).then_inc(s_sink, 16)
                for b in range(B):
                    sync.dma_start(xt[b][:], xs_in[b]).then_inc(s_x[b], 16)
                if use_collective:
                    # AllReduce payload on SP's queue ahead of the tin
                    # stream: its completion can't be delayed by prefetch
                    sync.wait_ge(s_colsum_sb, 1)
                    sync.dma_start(cc_in[:], colsum_sb[:]).then_inc(
                        s_cci, Q_CC)
                for k in range(NG):
                    # keep at most THROTTLE transfers queued so the gate
                    # phase's small DMAs aren't stuck behind bulk prefetch
                    if k >= THROTTLE:
                        j = k - THROTTLE
                        sync.wait_ge(s_in_slot[j % RING],
                                     Q_IN * (j // RING + 1))
                    if use_collective and k == PAUSE_K:
                        # pause once mid-stream so the post-AllReduce wbar
                        # broadcast isn't queued behind the whole prefetch
                        sync.wait_ge(s_ws, 16)
                    if k >= RING:
                        sync.wait_ge(s_out_slot[k % RING],
                                     Q_OUT * (k // RING))
                    sync.dma_start(
                        tin[k % RING][:],
                        attn_in[k * GROUP:(k + 1) * GROUP].rearrange(
                            "g p t -> p g t"),
                    ).then_inc(s_in_slot[k % RING], Q_IN)

            @block.gpsimd
            def _(gpsimd):
                if use_collective:
                    gpsimd.wait_ge(s_cci, Q_CC)
                    gpsimd.collective_compute(
                        "AllReduce",
                        ALU.add,
                        replica_groups=[list(range(N_CORES))],
                        ins=[cc_in[:]],
                        outs=[cc_out[:]],
                    ).then_inc(s_cc, 1)
                    gpsimd.wait_ge(s_cc, 1)
                    gpsimd.dma_start(
                        wbar_sb[:], cc_out[:].broadcast_to((128, D))
                    ).then_inc(s_ws, 16)
                if not use_collective:
                    gpsimd.dma_start(
                        wbar_sb[:], wsum_in[:].broadcast_to((128, D))
                    ).then_inc(s_ws, 16)
                gpsimd.wait_ge(s_staging, 1)
                gpsimd.dma_start(extras_dram[:], staging[:]).then_inc(s_exo, Q_EX)
                gpsimd.wait_ge(s_exo, Q_EX)
                gpsimd.dma_start(
                    extras_sb[:], extras_dram[:].broadcast_to((128, 4))
                ).then_inc(s_exb, Q_EXB)

            if use_collective:
                @block.tensor
                def _(tensor):
                    # colsum of local Wd rows: accumulate both row-tiles
                    tensor.wait_ge(s_ones, 1)
                    tensor.wait_ge(s_wd[0], 16)
                    for ni in range(D // 512):
                        nc.tensor.matmul(
                            colsum_ps[:, ni * 512:(ni + 1) * 512],
                            lhsT=ones_col[:],
                            rhs=wd_t[0][:, ni * 512:(ni + 1) * 512],
                            start=True, stop=False)
                    tensor.wait_ge(s_wd[1], 16)
                    for ni in range(D // 512):
                        mm = nc.tensor.matmul(
                            colsum_ps[:, ni * 512:(ni + 1) * 512],
                            lhsT=ones_col[:],
                            rhs=wd_t[1][:, ni * 512:(ni + 1) * 512],
                            start=False, stop=True)
                    mm.then_inc(s_pe, 1)

            @block.vector
            def _(vector):
                vc = 0
                if use_collective:
                    nc.vector.memset(ones_col[:], 1.0).then_inc(s_ones, 1)
                    vector.wait_ge(s_pe, 1)
                    nc.vector.tensor_copy(
                        colsum_sb[:], colsum_ps[:]).then_inc(s_colsum_sb, 1)
                # staging = [gW00, gW01, gb, mean(bd)+EPS]
                vector.wait_ge(s_x[0], 16)  # covers bd+gwb via queue FIFO
                vector.wait_ge(s_dinit, 1)
                nc.vector.tensor_copy(staging[:, 0:3], gwb_sb[:])
                nc.vector.tensor_copy(
                    staging[:, 3:4], dinit[:]).then_inc(s_staging, 1)
                # early reduces (collective mode only): the first RING
                # groups' row-sums depend only on their in-DMAs, so run them
                # while the AllReduce/gate chain is still in flight.  In host
                # mode the gate is ready long before the stream, so hoisting
                # would only delay it.
                n_hoist = N_HOIST if use_collective else 0
                for k in range(n_hoist):
                    vector.wait_ge(s_in_slot[k % RING], Q_IN * (k // RING + 1))
                    nc.vector.reduce_sum(
                        rs_all[:, k * GROUP:(k + 1) * GROUP],
                        tin[k % RING].rearrange("p (g t) -> p g t", g=GROUP),
                        axis=AX.X).then_inc(s_rs, 1)
                # he/gate chain, both b at once; every same-engine dependent
                # pair is completion-synced via s_vchain
                vector.wait_ge(s_exb, Q_EXB)
                vector.wait_ge(s_ws, 16)
                for b in range(B):
                    vector.wait_ge(s_x[b], 16)
                    nc.vector.tensor_mul(
                        xt[b][:], xt[b][:], wbar_sb[:]).then_inc(s_vchain, 1)
                vc += B; vector.wait_ge(s_vchain, vc)
                for b in range(B):
                    nc.vector.reduce_sum(
                        ghraw[:, b:b + 1], xt[b][:], axis=AX.X
                    ).then_inc(s_vchain, 1)
                vc += B; vector.wait_ge(s_vchain, vc)
                nc.vector.tensor_scalar(
                    out=dcol[:], in0=ghraw[:],
                    scalar1=INV_D, scalar2=extras_sb[:, 3:4],
                    op0=ALU.mult, op1=ALU.add).then_inc(s_vchain, 1)
                vc += 1; vector.wait_ge(s_vchain, vc)
                for b in range(B):
                    nc.vector.tensor_scalar(
                        out=gden[:, b:b + 1], in0=dcol[:, b:b + 1],
                        scalar1=dcol[:, b:b + 1], scalar2=0.01,
                        op0=ALU.mult, op1=ALU.add).then_inc(s_gden, 1)
                vector.wait_ge(s_grec, 1)
                nc.vector.tensor_mul(gt1[:], dcol[:], grec[:])
                nc.vector.tensor_scalar(
                    out=gt2g[:], in0=grec[:], scalar1=extras_sb[:, 1:2],
                    scalar2=0.1, op0=ALU.mult, op1=ALU.mult
                ).then_inc(s_vchain, 1)
                vc += 1; vector.wait_ge(s_vchain, vc)
                nc.vector.tensor_scalar(
                    out=gt1g[:], in0=gt1[:], scalar1=extras_sb[:, 0:1],
                    scalar2=None, op0=ALU.mult).then_inc(s_vchain, 1)
                vc += 1; vector.wait_ge(s_vchain, vc)
                nc.vector.tensor_add(glin[:], gt1g[:], gt2g[:]).then_inc(s_lin, 1)
                # bounce gate so ACT's scale operand is cross-engine
                vector.wait_ge(s_gate, 1)
                nc.vector.tensor_copy(gate_d[:], gate_sb[:]).then_inc(s_gated, 1)
                # main loop (reduces for k >= RING happen in-loop)
                for k in range(NG):
                    cols = slice(k * GROUP, (k + 1) * GROUP)
                    if k >= n_hoist:
                        vector.wait_ge(s_in_slot[k % RING],
                                       Q_IN * (k // RING + 1))
                        nc.vector.reduce_sum(
                            rs_all[:, cols],
                            tin[k % RING].rearrange("p (g t) -> p g t",
                                                    g=GROUP),
                            axis=AX.X).then_inc(s_rs, 1)
                    vector.wait_ge(s_sc, k + 1)
                    for g in range(GROUP):
                        sl = slice(g * S, (g + 1) * S)
                        mi = nc.vector.tensor_scalar(
                            out=tin[k % RING][:, sl],
                            in0=tin[k % RING][:, sl],
                            scalar1=sc_all[:, k * GROUP + g:k * GROUP + g + 1],
                            scalar2=None, op0=ALU.mult)
                    mi.then_inc(s_mul_dve, 1)
                if debug:
                    nc.vector.tensor_copy(dbg[:, 0:4], rs_all[:, 0:4])
                    nc.vector.tensor_copy(dbg[:, 4:8], den_all[:, 0:4])
                    nc.vector.tensor_copy(dbg[:, 8:12], rec_all[:, 0:4])
                    nc.vector.tensor_copy(dbg[:, 12:16], sc_all[:, 0:4])
                    nc.vector.tensor_copy(dbg[:, 16:18], gate_sb[:])
                    nc.vector.tensor_copy(dbg[:, 18:22], extras_sb[:])
                    nc.vector.tensor_copy(dbg[:, 22:24], dcol[:])
                    nc.vector.tensor_copy(dbg[:, 24:26], ghraw[:])
                    nc.vector.tensor_copy(dbg[:, 26:28], glin[:])
                    nc.vector.tensor_copy(dbg[:, 28:30], gden[:])
                    nc.vector.tensor_copy(
                        dbg[:, 30:32], grec[:]).then_inc(s_dbg, 1)

            @block.scalar
            def _(scalar):
                ac = 0
                scalar.wait_ge(s_x[0], 16)  # bd landed (queue FIFO)
                nc.scalar.activation(
                    bd_sb[:], bd_sb[:], ACT_F.Copy,
                    bias=EPS * INV_D, scale=INV_D, accum_out=dinit[:],
                ).then_inc(s_dinit, 1)
                # gate reciprocal: grec = exp(-ln(gden)), both b at once
                scalar.wait_ge(s_gden, B)
                nc.scalar.activation(
                    grscr[:], gden[:], ACT_F.Ln,
                    bias=0.0, scale=1.0).then_inc(s_achain, 1)
                ac += 1; scalar.wait_ge(s_achain, ac)
                nc.scalar.activation(
                    grec[:], grscr[:], ACT_F.Exp,
                    bias=0.0, scale=-1.0).then_inc(s_grec, 1)
                scalar.wait_ge(s_lin, 1)
                nc.scalar.activation(
                    gate_sb[:], glin[:], ACT_F.Sigmoid,
                    bias=extras_sb[:, 2:3], scale=1.0).then_inc(s_gate, 1)
                scalar.wait_ge(s_gated, 1)
                nb = 4 if use_collective else 0
                if nb:
                    # groups 0..3 share b=0 and have hoisted row-sums: one
                    # [128, 16] chain for all of them
                    cols = slice(0, nb * GROUP)
                    scalar.wait_ge(s_rs, nb)
                    nc.scalar.activation(
                        den_all[:, cols], rs_all[:, cols], ACT_F.Copy,
                        bias=EPS, scale=gate_d[:, 0:1]).then_inc(s_achain, 1)
                    ac += 1; scalar.wait_ge(s_achain, ac)
                    nc.scalar.activation(
                        rec_scr[:, cols], den_all[:, cols], ACT_F.Ln,
                        bias=0.0, scale=1.0).then_inc(s_achain, 1)
                    ac += 1; scalar.wait_ge(s_achain, ac)
                    nc.scalar.activation(
                        rec_all[:, cols], rec_scr[:, cols], ACT_F.Exp,
                        bias=0.0, scale=-1.0).then_inc(s_achain, 1)
                    ac += 1; scalar.wait_ge(s_achain, ac)
                    nc.scalar.activation(
                        sc_all[:, cols], rec_all[:, cols], ACT_F.Copy,
                        bias=0.0, scale=gate_d[:, 0:1]).then_inc(s_sc, nb)
                    for k in range(nb):
                        scalar.wait_ge(s_mul_dve, k + 1)
                        scalar.dma_start(
                            out_d[k * GROUP:(k + 1) * GROUP].rearrange(
                                "g p t -> p g t"),
                            tin[k % RING][:],
                        ).then_inc(s_out_slot[k % RING], Q_OUT)
                for k in range(nb, NG):
                    b = (k * GROUP) // H
                    cols = slice(k * GROUP, (k + 1) * GROUP)
                    scalar.wait_ge(s_rs, k + 1)
                    # den = rs*gate + EPS ; rec = exp(-ln(den)) ; sc = rec*gate
                    nc.scalar.activation(
                        den_all[:, cols], rs_all[:, cols], ACT_F.Copy,
                        bias=EPS, scale=gate_d[:, b:b + 1]).then_inc(s_achain, 1)
                    ac += 1; scalar.wait_ge(s_achain, ac)
                    nc.scalar.activation(
                        rec_scr[:, cols], den_all[:, cols], ACT_F.Ln,
                        bias=0.0, scale=1.0).then_inc(s_achain, 1)
                    ac += 1; scalar.wait_ge(s_achain, ac)
                    nc.scalar.activation(
                        rec_all[:, cols], rec_scr[:, cols], ACT_F.Exp,
                        bias=0.0, scale=-1.0).then_inc(s_achain, 1)
                    ac += 1; scalar.wait_ge(s_achain, ac)
                    nc.scalar.activation(
                        sc_all[:, cols], rec_all[:, cols], ACT_F.Copy,
                        bias=0.0, scale=gate_d[:, b:b + 1]).then_inc(s_sc, 1)
                    scalar.wait_ge(s_mul_dve, k + 1)
                    scalar.dma_start(
                        out_d[k * GROUP:(k + 1) * GROUP].rearrange(
                            "g p t -> p g t"),
                        tin[k % RING][:],
                    ).then_inc(s_out_slot[k % RING], Q_OUT)
                if debug:
                    scalar.wait_ge(s_dbg, 1)
                    scalar.dma_start(dbg_out[:], dbg[:]).then_inc(s_sink, 16)
    return nc


_NC_CACHE = {}


def _get_nc(use_collective: bool):
    if use_collective not in _NC_CACHE:
        _NC_CACHE[use_collective] = build_kernel(use_collective)
    return _NC_CACHE[use_collective]


def kernel(x, attention_weights, Wd, bd, Wsup, bsup, Wsub, bsub, gW, gb):
    """Full inputs in, full output out; shards internally across 8 cores."""
    global LAST_EXEC_NS, LAST_RESULTS
    x = np.ascontiguousarray(x, dtype=np.float32)
    attention_weights = np.ascontiguousarray(attention_weights, dtype=np.float32)
    Wd = np.ascontiguousarray(Wd, dtype=np.float32)
    bd_r = np.asarray(bd, dtype=np.float32).reshape(1, D)
    gwb = np.array([[np.float32(gW[0, 0]), np.float32(gW[0, 1]),
                     np.float32(gb[0])]], dtype=np.float32)

    use_collective = COLLECTIVE_MODE
    nc = _get_nc(use_collective)

    in_maps = []
    for k in range(N_CORES):
        sk = k * S_CHUNK
        m = {
            "attn": np.ascontiguousarray(
                attention_weights[:, :, sk:sk + S_CHUNK, :]
            ).reshape(BH, S_CHUNK, S),
            "xs": np.ascontiguousarray(x[:, sk:sk + S_CHUNK, :]),
            "bd": bd_r,
            "gwb": gwb,
        }
        if use_collective:
            rk = k * (D // N_CORES)
            m["wd"] = np.ascontiguousarray(Wd[rk:rk + D // N_CORES, :])
        else:
            m["wsum"] = Wd.sum(axis=0, dtype=np.float32).reshape(1, D)
        in_maps.append(m)

    res = run_bass_kernel_spmd(nc, in_maps, list(range(N_CORES)), trace=TRACE)
    LAST_EXEC_NS = res.exec_time_ns
    LAST_RESULTS = res
    out = np.empty((B, H, S, S), dtype=np.float32)
    for k in range(N_CORES):
        sk = k * S_CHUNK
        out[:, :, sk:sk + S_CHUNK, :] = res.results[k]["out"].reshape(
            B, H, S_CHUNK, S)
    return out



# revision 10
# speedup vs baseline: 1.2282x; 1.2282x over previous
"""Trainium2 Bass kernel for nn_BKCoreHyperbolicIntegration (8 NeuronCores).

Reference computation:
    he[b,s]  = mean_e( x[b,s,:] @ Wd[e,:] + bd[e] ) = x @ colmean(Wd) + mean(bd)
    G        = 1 / (he - (0 + 0.1j) + 1e-6)
    gate     = sigmoid(gW[0,0]*Re(G) + gW[0,1]*Im(G) + gb[0])
    gated    = attention_weights * gate[:, None, :, None]
    out      = gated / (gated.sum(-1, keepdims=True) + 1e-6)

Algebra used (all exact, no approximation):
  * mean_e(x @ Wd.T + bd) == x @ colmean(Wd) + mean(bd).
  * h0_super / h0_sub in the reference are dead code -> skipped.
  * With z = 0.1j and d := he + EPS:  Re G = d/(d^2+0.01), Im G = 0.1/(d^2+0.01).
  * out = attn*g / (g*rs + EPS) == attn / (rs + EPS/g), and for g = sigmoid(z),
    1/g = 1 + exp(-z).  So the gate enters only through the tiny per-row
    denominator bias cb = EPS*(1 + exp(-z)); no sigmoid and no second
    gate multiply are needed.

Sharding: the S (row) axis of attention_weights is split across the 8 cores
(core k owns rows [128k, 128k+128) for every b,h).  Wd is COLUMN-sharded:
core k loads Wd[:, 256k:256k+256) (host-relaid into PE-friendly tiles),
PE-reduces it to its exact colsum slice [1,256], and an 8-core AllGather
(no reduce pass, so ~half the modeled cost of AllReduce) assembles the full
[1,2048] column sum, which is broadcast per-partition for the DVE dot
product with x.

All bulk tensors are host-relaid so every DMA moves contiguous 8-16KB runs
per partition (128 descriptors) - descriptor-count overhead on the shared
DMA engines is ~1ns/descriptor, so small descriptors cost real time.

The attention stream uses 16 pieces of [128, 2*1024] (2 heads per piece),
all resident in SBUF (no ring reuse).  Small pieces bound how long the
gate-phase small DMAs (cc_in, wbar broadcast) can be stuck in the shared
DMA-engine FIFO behind bulk prefetch (in-flight depth 2 = ~6us).

Per piece: DVE row-sum reduce; ACT folds den = rs + cb into Ln, Exp gives
the reciprocal, ACT scales both head rows in place (scale operand bounced
through DVE so every scalar-port operand is produced by a different
engine), and the POOL engine issues the output DMA (keeps the ~1.7us
per-DMA sequencer cost off the ACT critical path).

Toolchain behaviors inherited from the validated baseline:
  * all semaphores explicit, 1-2 waits per instruction (compiler limit).
  * reciprocal = exp(-ln(x)) on ACT (InstReciprocal returns inf on HW).
  * same-engine dependent pairs completion-synced via chain semaphores;
    scalar-port operands produced by a different engine.
  * DMA completion quanta: [128,*] DMAs post 16, [1,*] post 32.
  * PE is warmed with two dummy matmuls so the colsum runs at full clock.

Engine roles:
  SP     streams wd/x/attention pieces in (16 resident pieces, throttle 2)
  PE     warmup + Wd column-sum (ones.T @ wd tiles, PSUM-accumulated)
  Pool   AllGather, stride-0 broadcasts, extras staging, ALL output DMAs
  DVE    row-sum reduces, he multiplies, gate chain, sc bounce copies
  ACT    he accumulation, Ln/Exp chains, in-place piece scaling
"""

from contextlib import ExitStack

import numpy as np

import concourse.bass as bass
from concourse import mybir
from concourse.bass_utils import run_bass_kernel_spmd

TRACE = False
LAST_EXEC_NS = None
LAST_RESULTS = None

F32 = mybir.dt.float32
AX = mybir.AxisListType
ALU = mybir.AluOpType
ACT_F = mybir.ActivationFunctionType

B, S, H, D = 2, 1024, 16, 2048
N_CORES = 8
S_CHUNK = S // N_CORES        # 128 attn rows per core
BH = B * H                    # 32
DSL = D // N_CORES            # 256 Wd columns per core
NWT = D // 128                # 16 row-tiles of the Wd column slice
PG = 2                        # heads per attention piece
NP = BH // PG                 # 16 pieces
NCH = NP // 2                 # 8 ACT chain batches ([128, 4] each)
EPS = 1e-6
INV_D = 1.0 / D
Q_IN = 16                     # [128,*] DMA completion quantum
Q_CC = 32                     # [1,*] DMA completion quantum
THROTTLE = 2                  # max in-flight attention in-DMAs
N_HOIST = 10                  # reduces hoisted before the gate chain


def build_kernel(debug: bool = False, detect_races: bool = True):
    nc = bass.Bass(detect_race_conditions=detect_races)
    attn_in = nc.declare_dram_parameter("attn", [NP, 128, PG * S], F32, isOutput=False)
    xs_in = nc.declare_dram_parameter("xs", [128, B * D], F32, isOutput=False)
    wdc_in = nc.declare_dram_parameter("wdc", [128, NWT * DSL], F32, isOutput=False)
    bd_in = nc.declare_dram_parameter("bd", [1, D], F32, isOutput=False)
    gwb_in = nc.declare_dram_parameter("gwb", [1, 3], F32, isOutput=False)
    out_d = nc.declare_dram_parameter("out", [NP, 128, PG * S], F32, isOutput=True)
    cc_in = nc.dram_tensor("cc_in", [1, DSL], F32)
    cc_out = nc.dram_tensor("cc_out", [1, D], F32, addr_space="Shared")
    extras_dram = nc.dram_tensor("extras_dram", [1, 4], F32)

    ctx = ExitStack()
    with ctx:
        sb = lambda shape, name: ctx.enter_context(
            nc.sbuf_tensor(name, shape, F32))
        sem = lambda name: ctx.enter_context(nc.semaphore(name))

        tin = [sb([128, PG * S], f"tin{i}") for i in range(NP)]
        wd_sb = sb([128, NWT * DSL], "wd_sb")
        xt = sb([128, B * D], "xt")
        wbar_sb = sb([128, D], "wbar_sb")
        bd_sb = sb([1, D], "bd_sb")
        gwb_sb = sb([1, 3], "gwb_sb")
        cs_sb = sb([1, DSL], "cs_sb")
        dinit = sb([1, 1], "dinit")
        staging = sb([1, 4], "staging")
        extras_sb = sb([128, 4], "extras_sb")
        rs_all = sb([128, BH], "rs_all")
        rec_scr = sb([128, BH], "rec_scr")
        rec_all = sb([128, BH], "rec_all")
        sc_all = sb([128, BH], "sc_all")
        ghraw = sb([128, B], "ghraw")
        dcol = sb([128, B], "dcol")
        gden = sb([128, B], "gden")
        grscr = sb([128, B], "grscr")
        grec = sb([128, B], "grec")
        gt1 = sb([128, B], "gt1")
        gt1g = sb([128, B], "gt1g")
        gt2g = sb([128, B], "gt2g")
        etm = sb([128, B], "etm")
        cb = sb([128, B], "cb")
        ones_col = sb([128, 1], "ones_col")
        warm = sb([128, 512], "warm")
        colsum_ps = ctx.enter_context(nc.psum_tensor("colsum_ps", [1, DSL], F32))
        warm_ps = ctx.enter_context(nc.psum_tensor("warm_ps", [1, 512], F32))

        s_in = [sem(f"s_in{i}") for i in range(NP)]
        s_x = sem("s_x")
        s_wd = sem("s_wd")
        s_cci = sem("s_cci")
        s_cc = sem("s_cc")
        s_ws = sem("s_ws")
        s_exo = sem("s_exo")
        s_exb = sem("s_exb")
        s_ones = sem("s_ones")
        s_pe = sem("s_pe")
        s_colsum_sb = sem("s_colsum_sb")
        s_dinit = sem("s_dinit")
        s_staging = sem("s_staging")
        s_hemul = sem("s_hemul")
        s_ghr = sem("s_ghr")
        s_gden = sem("s_gden")
        s_grec = sem("s_grec")
        s_lin = sem("s_lin")
        s_et = sem("s_et")
        s_cb = sem("s_cb")
        s_rs = sem("s_rs")
        s_recact = sem("s_recact")
        s_sc = sem("s_sc")
        s_ma = sem("s_ma")
        s_vchain = sem("s_vchain")
        s_achain = sem("s_achain")
        s_sink = sem("s_sink")
        s_out = sem("s_out")

        with nc.Block() as block:

            @block.sync
            def _(sync):
                # smalls first: no direct waiters - covered via queue FIFO by
                # the wd DMA's full completion
                sync.dma_start(bd_sb[:], bd_in[:]).then_inc(s_sink, 16)
                sync.dma_start(gwb_sb[:], gwb_in[:]).then_inc(s_sink, 16)
                sync.dma_start(wd_sb[:], wdc_in[:]).then_inc(s_wd, 16)
                sync.dma_start(xt[:], xs_in[:]).then_inc(s_x, 16)
                for p in range(NP):
                    # keep at most THROTTLE transfers queued so the gate
                    # phase's small DMAs aren't stuck behind bulk prefetch
                    if p >= THROTTLE:
                        sync.wait_ge(s_in[p - THROTTLE], Q_IN)
                    sync.dma_start(
                        tin[p][:], attn_in[p]).then_inc(s_in[p], Q_IN)

            @block.tensor
            def _(tensor):
                # warm the PE clock (pstate ramps with continuous busy time)
                tensor.wait_ge(s_ones, 1)
                for _ in range(2):
                    nc.tensor.matmul(
                        warm_ps[:], lhsT=ones_col[:], rhs=warm[:],
                        start=True, stop=True)
                tensor.wait_ge(s_wd, 16)
                for t in range(NWT):
                    mm = nc.tensor.matmul(
                        colsum_ps[:],
                        lhsT=ones_col[:],
                        rhs=wd_sb[:, t * DSL:(t + 1) * DSL],
                        start=(t == 0), stop=(t == NWT - 1))
                mm.then_inc(s_pe, 1)

            @block.gpsimd
            def _(gpsimd):
                gpsimd.wait_ge(s_colsum_sb, 1)
                gpsimd.dma_start(cc_in[:], cs_sb[:]).then_inc(s_cci, Q_CC)
                gpsimd.wait_ge(s_cci, Q_CC)
                gpsimd.collective_compute(
                    "AllGather",
                    ALU.bypass,
                    replica_groups=[list(range(N_CORES))],
                    ins=[cc_in[:]],
                    outs=[cc_out[:]],
                ).then_inc(s_cc, 1)
                gpsimd.wait_ge(s_cc, 1)
                gpsimd.dma_start(
                    wbar_sb[:], cc_out[:].broadcast_to((128, D))
                ).then_inc(s_ws, 16)
                gpsimd.wait_ge(s_staging, 1)
                gpsimd.dma_start(extras_dram[:], staging[:]).then_inc(s_exo, 16)
                gpsimd.wait_ge(s_exo, 16)
                gpsimd.dma_start(
                    extras_sb[:], extras_dram[:].broadcast_to((128, 4))
                ).then_inc(s_exb, 16)
                # output stream: all piece scaling is done by ACT (s_ma)
                for p in range(NP):
                    gpsimd.wait_ge(s_ma, p + 1)
                    gpsimd.dma_start(out_d[p], tin[p][:]).then_inc(s_out, 16)

            @block.vector
            def _(vector):
                vc = 0
                nc.vector.memset(warm[:], 1.0)
                nc.vector.memset(ones_col[:], 1.0).then_inc(s_ones, 1)
                vector.wait_ge(s_pe, 1)
                nc.vector.tensor_copy(cs_sb[:], colsum_ps[:]).then_inc(
                    s_colsum_sb, 1)
                # staging = [gW00, gW01, -gb, mean(bd)+EPS]
                vector.wait_ge(s_wd, 16)  # covers bd+gwb via queue FIFO
                vector.wait_ge(s_dinit, 1)
                nc.vector.tensor_copy(staging[:, 0:2], gwb_sb[:, 0:2])
                nc.vector.tensor_scalar(
                    out=staging[:, 2:3], in0=gwb_sb[:, 2:3],
                    scalar1=-1.0, scalar2=None, op0=ALU.mult)
                nc.vector.tensor_copy(
                    staging[:, 3:4], dinit[:]).then_inc(s_staging, 1)
                # early reduces: row-sums depend only on their in-DMAs, so
                # run them while the AllGather / gate chain is in flight
                for p in range(N_HOIST):
                    vector.wait_ge(s_in[p], Q_IN)
                    nc.vector.reduce_sum(
                        rs_all[:, p * PG:(p + 1) * PG],
                        tin[p].rearrange("p (g t) -> p g t", g=PG),
                        axis=AX.X).then_inc(s_rs, 1)
                # he multiplies (ACT accumulates them into ghraw)
                vector.wait_ge(s_ws, 16)
                vector.wait_ge(s_x, 16)
                for b in range(B):
                    nc.vector.tensor_mul(
                        xt[:, b * D:(b + 1) * D], xt[:, b * D:(b + 1) * D],
                        wbar_sb[:]).then_inc(s_hemul, 1)
                # gate chain; same-engine dependent pairs completion-synced
                vector.wait_ge(s_ghr, B)
                vector.wait_ge(s_exb, 16)
                nc.vector.tensor_scalar(
                    out=dcol[:], in0=ghraw[:],
                    scalar1=INV_D, scalar2=extras_sb[:, 3:4],
                    op0=ALU.mult, op1=ALU.add).then_inc(s_vchain, 1)
                vc += 1; vector.wait_ge(s_vchain, vc)
                for b in range(B):
                    nc.vector.tensor_scalar(
                        out=gden[:, b:b + 1], in0=dcol[:, b:b + 1],
                        scalar1=dcol[:, b:b + 1], scalar2=0.01,
                        op0=ALU.mult, op1=ALU.add).then_inc(s_gden, 1)
                vector.wait_ge(s_grec, 1)
                nc.vector.tensor_mul(gt1[:], dcol[:], grec[:])
                nc.vector.tensor_scalar(
                    out=gt2g[:], in0=grec[:], scalar1=extras_sb[:, 1:2],
                    scalar2=0.1, op0=ALU.mult, op1=ALU.mult
                ).then_inc(s_vchain, 1)
                vc += 1; vector.wait_ge(s_vchain, vc)
                nc.vector.tensor_scalar(
                    out=gt1g[:], in0=gt1[:], scalar1=extras_sb[:, 0:1],
                    scalar2=None, op0=ALU.mult).then_inc(s_vchain, 1)
                vc += 1; vector.wait_ge(s_vchain, vc)
                nc.vector.tensor_add(gt1g[:], gt1g[:], gt2g[:]).then_inc(s_lin, 1)
                # cb = EPS * (1 + exp(-(lin+gb))) : the whole gate effect
                vector.wait_ge(s_et, 1)
                nc.vector.tensor_scalar(
                    out=cb[:], in0=etm[:], scalar1=EPS, scalar2=EPS,
                    op0=ALU.mult, op1=ALU.add).then_inc(s_cb, 1)
                # interleave sc bounce copies (ACT scale-port operands must
                # come from another engine) with the remaining reduces
                for q in range(NCH):
                    vector.wait_ge(s_recact, q + 1)
                    nc.vector.tensor_copy(
                        sc_all[:, q * 4:(q + 1) * 4],
                        rec_all[:, q * 4:(q + 1) * 4]).then_inc(s_sc, 1)
                    p = N_HOIST + q
                    if p < NP:
                        vector.wait_ge(s_in[p], Q_IN)
                        nc.vector.reduce_sum(
                            rs_all[:, p * PG:(p + 1) * PG],
                            tin[p].rearrange("p (g t) -> p g t", g=PG),
                            axis=AX.X).then_inc(s_rs, 1)

            @block.scalar
            def _(scalar):
                ac = 0
                scalar.wait_ge(s_wd, 16)  # bd landed (queue FIFO)
                nc.scalar.activation(
                    bd_sb[:], bd_sb[:], ACT_F.Copy,
                    bias=EPS * INV_D, scale=INV_D, accum_out=dinit[:],
                ).then_inc(s_dinit, 1)
                # he accumulation: ghraw[:, b] = sum_d xt[:, b*D:(b+1)*D]
                for b in range(B):
                    scalar.wait_ge(s_hemul, b + 1)
                    nc.scalar.activation(
                        xt[:, b * D:(b + 1) * D], xt[:, b * D:(b + 1) * D],
                        ACT_F.Copy, bias=0.0, scale=1.0,
                        accum_out=ghraw[:, b:b + 1]).then_inc(s_ghr, 1)
                # grec = 1/gden = exp(-ln(gden))
                scalar.wait_ge(s_gden, B)
                nc.scalar.activation(
                    grscr[:], gden[:], ACT_F.Ln,
                    bias=0.0, scale=1.0).then_inc(s_achain, 1)
                ac += 1; scalar.wait_ge(s_achain, ac)
                nc.scalar.activation(
                    grec[:], grscr[:], ACT_F.Exp,
                    bias=0.0, scale=-1.0).then_inc(s_grec, 1)
                # etm = exp(-(lin + gb))   (bias AP holds -gb)
                scalar.wait_ge(s_lin, 1)
                nc.scalar.activation(
                    etm[:], gt1g[:], ACT_F.Exp,
                    bias=extras_sb[:, 2:3], scale=-1.0).then_inc(s_et, 1)
                # per chain batch q (pieces 2q, 2q+1; 4 bh columns):
                #   rec = exp(-ln(rs + cb)) ; scale both pieces in place
                for q in range(NCH):
                    b = q // (NCH // B)
                    cols = slice(q * 4, (q + 1) * 4)
                    if q == 0:
                        scalar.wait_ge(s_cb, 1)
                    scalar.wait_ge(s_rs, 2 * q + 2)
                    nc.scalar.activation(
                        rec_scr[:, cols], rs_all[:, cols], ACT_F.Ln,
                        bias=cb[:, b:b + 1], scale=1.0).then_inc(s_achain, 1)
                    ac += 1; scalar.wait_ge(s_achain, ac)
                    nc.scalar.activation(
                        rec_all[:, cols], rec_scr[:, cols], ACT_F.Exp,
                        bias=0.0, scale=-1.0).then_inc(s_recact, 1)
                    for pp in range(2):
                        p = 2 * q + pp
                        if pp == 0:
                            scalar.wait_ge(s_sc, q + 1)
                        for g in range(PG):
                            col = p * PG + g
                            mi = nc.scalar.activation(
                                tin[p][:, g * S:(g + 1) * S],
                                tin[p][:, g * S:(g + 1) * S], ACT_F.Copy,
                                bias=0.0, scale=sc_all[:, col:col + 1])
                        mi.then_inc(s_ma, 1)
    return nc


_NC_CACHE = {}


def _get_nc():
    if "nc" not in _NC_CACHE:
        _NC_CACHE["nc"] = build_kernel()
    return _NC_CACHE["nc"]


def kernel(x, attention_weights, Wd, bd, Wsup, bsup, Wsub, bsub, gW, gb):
    """Full inputs in, full output out; shards internally across 8 cores."""
    global LAST_EXEC_NS, LAST_RESULTS
    x = np.ascontiguousarray(x, dtype=np.float32)
    attention_weights = np.ascontiguousarray(attention_weights, dtype=np.float32)
    Wd = np.ascontiguousarray(Wd, dtype=np.float32)
    bd_r = np.asarray(bd, dtype=np.float32).reshape(1, D)
    gwb = np.array([[np.float32(gW[0, 0]), np.float32(gW[0, 1]),
                     np.float32(gb[0])]], dtype=np.float32)

    nc = _get_nc()

    in_maps = []
    for k in range(N_CORES):
        sk = k * S_CHUNK
        ck = k * DSL
        # attn: [NP, 128, PG*S] with tin[p][s, g*S+t] = attn[bh=p*PG+g, s, t]
        attn_relay = np.ascontiguousarray(
            attention_weights[:, :, sk:sk + S_CHUNK, :]
            .reshape(NP, PG, S_CHUNK, S)
            .transpose(0, 2, 1, 3)
            .reshape(NP, 128, PG * S))
        # x: [128, B*D] with xt[s, b*D+d] = x[b, sk+s, d]
        xs_relay = np.ascontiguousarray(
            x[:, sk:sk + S_CHUNK, :].transpose(1, 0, 2).reshape(128, B * D))
        # wd: [128, NWT*DSL] with wd_sb[p, t*DSL+c] = Wd[t*128+p, ck+c]
        wd_relay = np.ascontiguousarray(
            Wd[:, ck:ck + DSL].reshape(NWT, 128, DSL)
            .transpose(1, 0, 2).reshape(128, NWT * DSL))
        in_maps.append({
            "attn": attn_relay,
            "xs": xs_relay,
            "wdc": wd_relay,
            "bd": bd_r,
            "gwb": gwb,
        })

    res = run_bass_kernel_spmd(nc, in_maps, list(range(N_CORES)), trace=TRACE)
    LAST_EXEC_NS = res.exec_time_ns
    LAST_RESULTS = res
    out = np.empty((B, H, S, S), dtype=np.float32)
    for k in range(N_CORES):
        sk = k * S_CHUNK
        out[:, :, sk:sk + S_CHUNK, :] = (
            res.results[k]["out"]
            .reshape(NP, S_CHUNK, PG, S)
            .transpose(0, 2, 1, 3)
            .reshape(B, H, S_CHUNK, S))
    return out


# revision 31
# speedup vs baseline: 1.6322x; 1.3289x over previous
"""Trainium2 Bass kernel for nn_BKCoreHyperbolicIntegration (8 NeuronCores).

Reference computation:
    he[b,s]  = mean_e( x[b,s,:] @ Wd[e,:] + bd[e] ) = x @ colmean(Wd) + mean(bd)
    G        = 1 / (he - (0 + 0.1j) + 1e-6)
    gate     = sigmoid(gW[0,0]*Re(G) + gW[0,1]*Im(G) + gb[0])
    gated    = attention_weights * gate[:, None, :, None]
    out      = gated / (gated.sum(-1, keepdims=True) + 1e-6)

Algebra used (all exact, no approximation):
  * mean_e(x @ Wd.T + bd) == x @ colmean(Wd) + mean(bd).
  * h0_super / h0_sub in the reference are dead code -> skipped.
  * With z = 0.1j and d := he + EPS:  Re G = d/(d^2+0.01), Im G = 0.1/(d^2+0.01).
  * out = attn*g / (g*rs + EPS) == attn / (rs + EPS/g), and for g = sigmoid(z),
    1/g = 1 + exp(-z).  So the gate enters only through the tiny per-row
    denominator bias cb = EPS*(1 + exp(-z)); no sigmoid and no second
    gate multiply are needed.

Sharding: the S (row) axis of attention_weights is split across the 8 cores
(core k owns rows [128k, 128k+128) for every b,h).  Wd is COLUMN-sharded:
core k loads Wd[:, 256k:256k+256) (host-relaid into PE-friendly tiles),
PE-reduces it to its exact colsum slice [1,256], and an 8-core AllGather
(no reduce pass, so ~half the modeled cost of AllReduce) assembles the full
[1,2048] column sum, which is broadcast per-partition for the DVE dot
product with x.

All bulk tensors are host-relaid so every DMA moves contiguous 8-16KB runs
per partition (128 descriptors) - descriptor-count overhead on the shared
DMA engines is ~1ns/descriptor, so small descriptors cost real time.

The attention stream uses 16 pieces of [128, 2*1024] (2 heads per piece),
all resident in SBUF (no ring reuse).  Small pieces bound how long the
gate-phase small DMAs (cc_in, wbar broadcast) can be stuck in the shared
DMA-engine FIFO behind bulk prefetch (in-flight depth 2 = ~6us).

Per piece: DVE row-sum reduce; ACT folds den = rs + cb into Ln, Exp gives
the reciprocal, ACT scales both head rows in place (scale operand bounced
through DVE so every scalar-port operand is produced by a different
engine), and the POOL engine issues the output DMA (keeps the ~1.7us
per-DMA sequencer cost off the ACT critical path).

Toolchain behaviors inherited from the validated baseline:
  * all semaphores explicit, 1-2 waits per instruction (compiler limit).
  * reciprocal = exp(-ln(x)) on ACT (InstReciprocal returns inf on HW).
  * same-engine dependent pairs completion-synced via chain semaphores;
    scalar-port operands produced by a different engine.
  * DMA completion quanta: [128,*] DMAs post 16, [1,*] post 32.
  * PE is warmed with two dummy matmuls so the colsum runs at full clock.

Engine roles:
  SP     streams wd/x/attention pieces in (16 resident pieces, throttle 2)
  PE     warmup + Wd column-sum (ones.T @ wd tiles, PSUM-accumulated)
  Pool   AllGather, stride-0 broadcasts, extras staging, ALL output DMAs
  DVE    row-sum reduces, he multiplies, gate chain, sc bounce copies
  ACT    he accumulation, Ln/Exp chains, in-place piece scaling
"""

from contextlib import ExitStack

import numpy as np

import concourse.bass as bass
from concourse import library_config, mybir
from concourse.bass_utils import run_bass_kernel_spmd

TRACE = False
LAST_EXEC_NS = None
LAST_RESULTS = None

F32 = mybir.dt.float32
AX = mybir.AxisListType
ALU = mybir.AluOpType
ACT_F = mybir.ActivationFunctionType

B, S, H, D = 2, 1024, 16, 2048
N_CORES = 8
S_CHUNK = S // N_CORES        # 128 attn rows per core
BH = B * H                    # 32
DSL = D // N_CORES            # 256 Wd columns per core
NWT = D // 128                # 16 row-tiles of the Wd column slice
PG = 2                        # heads per attention piece
NP = BH // PG                 # 16 pieces
NCH = NP // 2                 # 8 ACT chain batches ([128, 4] each)
EPS = 1e-6
INV_D = 1.0 / D
Q_IN = 16                     # [128,*] DMA completion quantum
Q_CC = 32                     # [1,*] DMA completion quantum
THROTTLE = 3                  # max in-flight attention in-DMAs
N_HOIST = 10                  # reduces hoisted before the gate chain


def build_kernel(debug: bool = False, detect_races: bool = True):
    nc = bass.Bass(detect_race_conditions=detect_races)
    attn_in = nc.declare_dram_parameter("attn", [NP, 128, PG * S], F32, isOutput=False)
    xs_in = nc.declare_dram_parameter("xs", [128, B * D], F32, isOutput=False)
    wdc_in = nc.declare_dram_parameter("wdc", [128, NWT * DSL], F32, isOutput=False)
    bd_in = nc.declare_dram_parameter("bd", [1, D], F32, isOutput=False)
    gwb_in = nc.declare_dram_parameter("gwb", [1, 3], F32, isOutput=False)
    out_d = nc.declare_dram_parameter("out", [NP, 128, PG * S], F32, isOutput=True)
    cc_in = nc.dram_tensor("cc_in", [1, DSL], F32)
    cc_out = nc.dram_tensor("cc_out", [1, D], F32, addr_space="Shared")
    extras_dram = nc.dram_tensor("extras_dram", [1, 4], F32)

    ctx = ExitStack()
    with ctx:
        sb = lambda shape, name: ctx.enter_context(
            nc.sbuf_tensor(name, shape, F32))
        sem = lambda name: ctx.enter_context(nc.semaphore(name))

        tin = [sb([128, PG * S], f"tin{i}") for i in range(NP)]
        wd_sb = sb([128, NWT * DSL], "wd_sb")
        xt = sb([128, B * D], "xt")
        wbar_sb = sb([128, D], "wbar_sb")
        bd_sb = sb([1, D], "bd_sb")
        gwb_sb = sb([1, 3], "gwb_sb")
        cs_sb = sb([1, DSL], "cs_sb")
        cc_sb = sb([1, D], "cc_sb")
        dinit = sb([1, 1], "dinit")
        staging = sb([1, 4], "staging")
        extras_sb = sb([128, 4], "extras_sb")
        rs_all = sb([128, BH], "rs_all")
        rec_scr = sb([128, BH], "rec_scr")
        rec_all = sb([128, BH], "rec_all")
        sc_all = sb([128, BH], "sc_all")
        ghraw = sb([128, B], "ghraw")
        dcol = sb([128, B], "dcol")
        gden = sb([128, B], "gden")
        grscr = sb([128, B], "grscr")
        grec = sb([128, B], "grec")
        gt1 = sb([128, B], "gt1")
        gt1g = sb([128, B], "gt1g")
        gt2g = sb([128, B], "gt2g")
        etm = sb([128, B], "etm")
        cb = sb([128, B], "cb")
        ones_col = sb([128, 1], "ones_col")
        c001 = sb([128, 1], "c001")
        warm = sb([128, 512], "warm")
        colsum_ps = ctx.enter_context(nc.psum_tensor("colsum_ps", [1, DSL], F32))
        warm_ps = ctx.enter_context(nc.psum_tensor("warm_ps", [1, 512], F32))

        s_in = [sem(f"s_in{i}") for i in range(NP)]
        s_x = sem("s_x")
        s_wd = sem("s_wd")
        s_wd2 = sem("s_wd2")
        s_rs_d = sem("s_rs_d")
        s_rs_a = sem("s_rs_a")
        s_md = sem("s_md")
        s_cci = sem("s_cci")
        s_cc = sem("s_cc")
        s_ws = sem("s_ws")
        s_exo = sem("s_exo")
        s_exb = sem("s_exb")
        s_ones = sem("s_ones")
        s_pe = sem("s_pe")
        s_colsum_sb = sem("s_colsum_sb")
        s_dinit = sem("s_dinit")
        s_staging = sem("s_staging")
        s_hemul = sem("s_hemul")
        s_ghr = sem("s_ghr")
        s_gden = sem("s_gden")
        s_grec = sem("s_grec")
        s_lin = sem("s_lin")
        s_et = sem("s_et")
        s_cb = sem("s_cb")
        s_rs = sem("s_rs")
        s_recact = sem("s_recact")
        s_sc = sem("s_sc")
        s_ma = sem("s_ma")
        s_vchain = sem("s_vchain")
        s_achain = sem("s_achain")
        s_sink = sem("s_sink")
        s_out = sem("s_out")

        with nc.Block() as block:

            # piece ownership per DMA queue (ins) and reduce engine
            IN_ACT = list(range(0, 7))      # ACT queue: pieces 0-6 (early)
            IN_SP = list(range(7, 12))      # SP queue after wd/xs prefix
            IN_POOL = list(range(12, 16))   # Pool queue, during the AllGather
            RED_D = [0, 1, 2, 3, 8, 9, 14, 10, 15, 11]   # DVE, arrival order
            RED_A = [7, 4, 12, 5, 13, 6]                 # ACT accum reduces
            # chain batch q -> required (s_rs_d, s_rs_a) counts
            CHAIN_ND = {q: max([RED_D.index(p) + 1 for p in (2 * q, 2 * q + 1)
                                if p in RED_D], default=0) for q in range(NCH)}
            CHAIN_NA = {q: max([RED_A.index(p) + 1 for p in (2 * q, 2 * q + 1)
                                if p in RED_A], default=0) for q in range(NCH)}
            HWT = NWT // 2

            @block.sync
            def _(sync):
                # wd first: it gates colsum -> AllGather -> gate.  bd/gwb
                # have no direct waiters - covered via queue FIFO by in7.
                sync.dma_start(
                    wd_sb[:, :HWT * DSL], wdc_in[:, :HWT * DSL]
                ).then_inc(s_wd, 16)
                sync.dma_start(
                    wd_sb[:, HWT * DSL:], wdc_in[:, HWT * DSL:]
                ).then_inc(s_wd2, 16)
                sync.dma_start(xt[:], xs_in[:]).then_inc(s_x, 16)
                sync.dma_start(bd_sb[:], bd_in[:]).then_inc(s_sink, 16)
                sync.dma_start(gwb_sb[:], gwb_in[:]).then_inc(s_sink, 16)
                for p in IN_SP:
                    sync.dma_start(
                        tin[p][:], attn_in[p]).then_inc(s_in[p], Q_IN)
                for p in range(0, NP, 2):
                    sync.wait_ge(s_ma, p + 1)
                    sync.wait_ge(s_md, p + 1)
                    sync.dma_start(out_d[p], tin[p][:]).then_inc(s_sink, 16)

            @block.tensor
            def _(tensor):
                # warm the PE clock (pstate ramps with continuous busy time)
                tensor.wait_ge(s_ones, 1)
                for _ in range(2):
                    nc.tensor.matmul(
                        warm_ps[:], lhsT=ones_col[:], rhs=warm[:],
                        start=True, stop=True)
                tensor.wait_ge(s_wd, 16)
                for t in range(HWT):
                    nc.tensor.matmul(
                        colsum_ps[:],
                        lhsT=ones_col[:],
                        rhs=wd_sb[:, t * DSL:(t + 1) * DSL],
                        start=(t == 0), stop=False)
                tensor.wait_ge(s_wd2, 16)
                for t in range(HWT, NWT):
                    mm = nc.tensor.matmul(
                        colsum_ps[:],
                        lhsT=ones_col[:],
                        rhs=wd_sb[:, t * DSL:(t + 1) * DSL],
                        start=False, stop=(t == NWT - 1))
                mm.then_inc(s_pe, 1)

            @block.gpsimd
            def _(gpsimd):
                gpsimd.wait_ge(s_colsum_sb, 1)
                gpsimd.dma_start(cc_in[:], cs_sb[:]).then_inc(s_cci, Q_CC)
                gpsimd.wait_ge(s_cci, Q_CC)
                gpsimd.collective_compute(
                    "AllGather",
                    ALU.bypass,
                    replica_groups=[list(range(N_CORES))],
                    ins=[cc_in[:]],
                    outs=[cc_out[:]],
                ).then_inc(s_cc, 1)
                # these transfers run while the AllGather is in flight
                for p in IN_POOL:
                    gpsimd.dma_start(
                        tin[p][:], attn_in[p]).then_inc(s_in[p], Q_IN)
                gpsimd.wait_ge(s_cc, 1)
                gpsimd.dma_start(
                    wbar_sb[:], cc_out[:].broadcast_to((128, D))
                ).then_inc(s_ws, 16)
                gpsimd.wait_ge(s_staging, 1)
                gpsimd.dma_start(extras_dram[:], staging[:]).then_inc(s_exo, 16)
                gpsimd.wait_ge(s_exo, 16)
                gpsimd.dma_start(
                    extras_sb[:], extras_dram[:].broadcast_to((128, 4))
                ).then_inc(s_exb, 16)
                for p in range(1, NP, 2):
                    gpsimd.wait_ge(s_ma, p + 1)
                    gpsimd.wait_ge(s_md, p + 1)
                    gpsimd.dma_start(out_d[p], tin[p][:]).then_inc(s_out, 16)

            @block.vector
            def _(vector):
                vc = 0
                nc.vector.memset(warm[:], 1.0)
                nc.vector.memset(c001[:], 0.01)
                nc.vector.memset(ones_col[:], 1.0).then_inc(s_ones, 1)
                vector.wait_ge(s_pe, 1)
                nc.vector.tensor_copy(cs_sb[:], colsum_ps[:]).then_inc(
                    s_colsum_sb, 1)
                # staging = [gW00, gW01, -gb, mean(bd)+EPS]
                vector.wait_ge(s_in[7], Q_IN)  # bd+gwb landed (queue FIFO)
                vector.wait_ge(s_dinit, 1)
                nc.vector.tensor_copy(staging[:, 0:2], gwb_sb[:, 0:2])
                nc.vector.tensor_scalar(
                    out=staging[:, 2:3], in0=gwb_sb[:, 2:3],
                    scalar1=-1.0, scalar2=None, op0=ALU.mult)
                nc.vector.tensor_copy(
                    staging[:, 3:4], dinit[:]).then_inc(s_staging, 1)
                # row-sum reduces, ordered by expected piece arrival
                for p in RED_D:
                    vector.wait_ge(s_in[p], Q_IN)
                    nc.vector.reduce_sum(
                        rs_all[:, p * PG:(p + 1) * PG],
                        tin[p].rearrange("p (g t) -> p g t", g=PG),
                        axis=AX.X).then_inc(s_rs_d, 1)
                # he multiplies (ACT accumulates them into ghraw)
                vector.wait_ge(s_ws, 16)
                vector.wait_ge(s_x, 16)
                for b in range(B):
                    nc.vector.tensor_mul(
                        xt[:, b * D:(b + 1) * D], xt[:, b * D:(b + 1) * D],
                        wbar_sb[:]).then_inc(s_hemul, 1)
                # gate chain; same-engine dependent pairs completion-synced
                vector.wait_ge(s_ghr, B)
                vector.wait_ge(s_exb, 16)
                nc.vector.tensor_scalar(
                    out=dcol[:], in0=ghraw[:],
                    scalar1=INV_D, scalar2=extras_sb[:, 3:4],
                    op0=ALU.mult, op1=ALU.add).then_inc(s_vchain, 1)
                vc += 1; vector.wait_ge(s_vchain, vc)
                nc.vector.tensor_mul(gden[:], dcol[:], dcol[:]).then_inc(
                    s_gden, 1)
                vector.wait_ge(s_grec, 1)
                nc.vector.tensor_mul(gt1[:], dcol[:], grec[:])
                nc.vector.tensor_scalar(
                    out=gt2g[:], in0=grec[:], scalar1=extras_sb[:, 1:2],
                    scalar2=0.1, op0=ALU.mult, op1=ALU.mult
                ).then_inc(s_vchain, 1)
                vc += 1; vector.wait_ge(s_vchain, vc)
                nc.vector.tensor_scalar(
                    out=gt1g[:], in0=gt1[:], scalar1=extras_sb[:, 0:1],
                    scalar2=None, op0=ALU.mult).then_inc(s_vchain, 1)
                vc += 1; vector.wait_ge(s_vchain, vc)
                nc.vector.tensor_add(gt1g[:], gt1g[:], gt2g[:]).then_inc(s_lin, 1)
                # cb = EPS * (1 + exp(-(lin+gb))) : the whole gate effect
                vector.wait_ge(s_et, 1)
                nc.vector.tensor_scalar(
                    out=cb[:], in0=etm[:], scalar1=EPS, scalar2=EPS,
                    op0=ALU.mult, op1=ALU.add).then_inc(s_cb, 1)
                # per batch: bounce sc (cross-engine scalar port), scale g=0
                for q in range(NCH):
                    vector.wait_ge(s_recact, q + 1)
                    nc.vector.tensor_copy(
                        sc_all[:, q * 4:(q + 1) * 4],
                        rec_all[:, q * 4:(q + 1) * 4]).then_inc(s_sc, 1)
                    vector.wait_ge(s_sc, q + 1)
                    for pp in range(2):
                        p = 2 * q + pp
                        col = p * PG
                        nc.vector.tensor_scalar(
                            out=tin[p][:, 0:S], in0=tin[p][:, 0:S],
                            scalar1=sc_all[:, col:col + 1], scalar2=None,
                            op0=ALU.mult).then_inc(s_md, 1)

            @block.scalar
            def _(scalar):
                ac = 0
                for p in IN_ACT:
                    scalar.dma_start(
                        tin[p][:], attn_in[p]).then_inc(s_in[p], Q_IN)
                scalar.wait_ge(s_in[7], Q_IN)  # bd landed (queue FIFO)
                nc.scalar.activation(
                    bd_sb[:], bd_sb[:], ACT_F.Copy,
                    bias=EPS * INV_D, scale=INV_D, accum_out=dinit[:],
                ).then_inc(s_dinit, 1)
                # accum-reduces for the ACT-owned pieces
                for p in RED_A:
                    scalar.wait_ge(s_in[p], Q_IN)
                    for g in range(PG):
                        mi = nc.scalar.activation(
                            tin[p][:, g * S:(g + 1) * S],
                            tin[p][:, g * S:(g + 1) * S], ACT_F.Copy,
                            bias=0.0, scale=1.0,
                            accum_out=rs_all[:, p * PG + g:p * PG + g + 1])
                    mi.then_inc(s_rs_a, 1)
                # he accumulation: ghraw[:, b] = sum_d xt[:, b*D:(b+1)*D]
                for b in range(B):
                    scalar.wait_ge(s_hemul, b + 1)
                    nc.scalar.activation(
                        xt[:, b * D:(b + 1) * D], xt[:, b * D:(b + 1) * D],
                        ACT_F.Copy, bias=0.0, scale=1.0,
                        accum_out=ghraw[:, b:b + 1]).then_inc(s_ghr, 1)
                # grec = 1/(dcol^2 + 0.01) = exp(-ln(gden + 0.01))
                scalar.wait_ge(s_gden, 1)
                nc.scalar.activation(
                    grscr[:], gden[:], ACT_F.Ln,
                    bias=c001[:], scale=1.0).then_inc(s_achain, 1)
                ac += 1; scalar.wait_ge(s_achain, ac)
                nc.scalar.activation(
                    grec[:], grscr[:], ACT_F.Exp,
                    bias=0.0, scale=-1.0).then_inc(s_grec, 1)
                # etm = exp(-(lin + gb))   (bias AP holds -gb)
                scalar.wait_ge(s_lin, 1)
                nc.scalar.activation(
                    etm[:], gt1g[:], ACT_F.Exp,
                    bias=extras_sb[:, 2:3], scale=-1.0).then_inc(s_et, 1)
                # per chain batch q (pieces 2q, 2q+1; 4 bh columns):
                #   rec = exp(-ln(rs + cb)) ; scale g=1 rows in place
                scalar.wait_ge(s_cb, 1)
                for q in range(NCH):
                    b = q // (NCH // B)
                    cols = slice(q * 4, (q + 1) * 4)
                    if CHAIN_ND[q]:
                        scalar.wait_ge(s_rs_d, CHAIN_ND[q])
                    if CHAIN_NA[q]:
                        scalar.wait_ge(s_rs_a, CHAIN_NA[q])
                    nc.scalar.activation(
                        rec_scr[:, cols], rs_all[:, cols], ACT_F.Ln,
                        bias=cb[:, b:b + 1], scale=1.0).then_inc(s_achain, 1)
                    ac += 1; scalar.wait_ge(s_achain, ac)
                    nc.scalar.activation(
                        rec_all[:, cols], rec_scr[:, cols], ACT_F.Exp,
                        bias=0.0, scale=-1.0).then_inc(s_recact, 1)
                    for pp in range(2):
                        p = 2 * q + pp
                        col = p * PG + 1
                        if pp == 0:
                            scalar.wait_ge(s_sc, q + 1)
                        nc.scalar.activation(
                            tin[p][:, S:2 * S],
                            tin[p][:, S:2 * S], ACT_F.Copy,
                            bias=0.0, scale=sc_all[:, col:col + 1]
                        ).then_inc(s_ma, 1)
    return nc


_NC_CACHE = {}


def _get_nc():
    if "nc" not in _NC_CACHE:
        _NC_CACHE["nc"] = build_kernel()
    return _NC_CACHE["nc"]


def kernel(x, attention_weights, Wd, bd, Wsup, bsup, Wsub, bsub, gW, gb):
    """Full inputs in, full output out; shards internally across 8 cores."""
    global LAST_EXEC_NS, LAST_RESULTS
    x = np.ascontiguousarray(x, dtype=np.float32)
    attention_weights = np.ascontiguousarray(attention_weights, dtype=np.float32)
    Wd = np.ascontiguousarray(Wd, dtype=np.float32)
    bd_r = np.asarray(bd, dtype=np.float32).reshape(1, D)
    gwb = np.array([[np.float32(gW[0, 0]), np.float32(gW[0, 1]),
                     np.float32(gb[0])]], dtype=np.float32)

    nc = _get_nc()

    in_maps = []
    for k in range(N_CORES):
        sk = k * S_CHUNK
        ck = k * DSL
        # attn: [NP, 128, PG*S] with tin[p][s, g*S+t] = attn[bh=p*PG+g, s, t]
        attn_relay = np.ascontiguousarray(
            attention_weights[:, :, sk:sk + S_CHUNK, :]
            .reshape(NP, PG, S_CHUNK, S)
            .transpose(0, 2, 1, 3)
            .reshape(NP, 128, PG * S))
        # x: [128, B*D] with xt[s, b*D+d] = x[b, sk+s, d]
        xs_relay = np.ascontiguousarray(
            x[:, sk:sk + S_CHUNK, :].transpose(1, 0, 2).reshape(128, B * D))
        # wd: [128, NWT*DSL] with wd_sb[p, t*DSL+c] = Wd[t*128+p, ck+c]
        wd_relay = np.ascontiguousarray(
            Wd[:, ck:ck + DSL].reshape(NWT, 128, DSL)
            .transpose(1, 0, 2).reshape(128, NWT * DSL))
        in_maps.append({
            "attn": attn_relay,
            "xs": xs_relay,
            "wdc": wd_relay,
            "bd": bd_r,
            "gwb": gwb,
        })

    res = run_bass_kernel_spmd(nc, in_maps, list(range(N_CORES)), trace=TRACE)
    LAST_EXEC_NS = res.exec_time_ns
    LAST_RESULTS = res
    out = np.empty((B, H, S, S), dtype=np.float32)
    for k in range(N_CORES):
        sk = k * S_CHUNK
        out[:, :, sk:sk + S_CHUNK, :] = (
            res.results[k]["out"]
            .reshape(NP, S_CHUNK, PG, S)
            .transpose(0, 2, 1, 3)
            .reshape(B, H, S_CHUNK, S))
    return out


# revision 37
# speedup vs baseline: 1.8957x; 1.1614x over previous
"""Trainium2 Bass kernel for nn_BKCoreHyperbolicIntegration (8 NeuronCores).

Reference computation:
    he[b,s]  = mean_e( x[b,s,:] @ Wd[e,:] + bd[e] ) = x @ colmean(Wd) + mean(bd)
    G        = 1 / (he - (0 + 0.1j) + 1e-6)
    gate     = sigmoid(gW[0,0]*Re(G) + gW[0,1]*Im(G) + gb[0])
    gated    = attention_weights * gate[:, None, :, None]
    out      = gated / (gated.sum(-1, keepdims=True) + 1e-6)

Algebra used (all exact, no approximation):
  * mean_e(x @ Wd.T + bd) == x @ colmean(Wd) + mean(bd).
  * h0_super / h0_sub in the reference are dead code -> skipped.
  * With z = 0.1j and d := he + EPS:  Re G = d/(d^2+0.01), Im G = 0.1/(d^2+0.01).
  * out = attn*g / (g*rs + EPS) == attn / (rs + EPS/g), and for g = sigmoid(z),
    1/g = 1 + exp(-z).  So the gate enters only through the tiny per-row
    denominator bias cb = EPS*(1 + exp(-z)); no sigmoid and no second
    gate multiply are needed.

Sharding: the S (row) axis of attention_weights is split across the 8 cores
(core k owns rows [128k, 128k+128) for every b,h).  Wd is COLUMN-sharded:
core k loads Wd[:, 256k:256k+256) (host-relaid into PE-friendly tiles),
PE-reduces it to its exact colsum slice [1,256], and an 8-core AllGather
(no reduce pass, so ~half the modeled cost of AllReduce) assembles the full
[1,2048] column sum, which is broadcast per-partition for the DVE dot
product with x.

All bulk tensors are host-relaid so every DMA moves contiguous 8-16KB runs
per partition (128 descriptors) - descriptor-count overhead on the shared
DMA engines is ~1ns/descriptor, so small descriptors cost real time.

The attention stream uses 16 pieces of [128, 2*1024] (2 heads per piece),
all resident in SBUF (no ring reuse).  Small pieces bound how long the
gate-phase small DMAs (cc_in, wbar broadcast) can be stuck in the shared
DMA-engine FIFO behind bulk prefetch (in-flight depth 2 = ~6us).

Per piece: DVE row-sum reduce; ACT folds den = rs + cb into Ln, Exp gives
the reciprocal, ACT scales both head rows in place (scale operand bounced
through DVE so every scalar-port operand is produced by a different
engine), and the POOL engine issues the output DMA (keeps the ~1.7us
per-DMA sequencer cost off the ACT critical path).

Toolchain behaviors inherited from the validated baseline:
  * all semaphores explicit, 1-2 waits per instruction (compiler limit).
  * reciprocal = exp(-ln(x)) on ACT (InstReciprocal returns inf on HW).
  * same-engine dependent pairs completion-synced via chain semaphores;
    scalar-port operands produced by a different engine.
  * DMA completion quanta: [128,*] DMAs post 16, [1,*] post 32.
  * PE is warmed with two dummy matmuls so the colsum runs at full clock.

Engine roles:
  SP     streams wd/x/attention pieces in (16 resident pieces, throttle 2)
  PE     warmup + Wd column-sum (ones.T @ wd tiles, PSUM-accumulated)
  Pool   AllGather, stride-0 broadcasts, extras staging, ALL output DMAs
  DVE    row-sum reduces, he multiplies, gate chain, sc bounce copies
  ACT    he accumulation, Ln/Exp chains, in-place piece scaling
"""

from contextlib import ExitStack

import numpy as np

import concourse.bass as bass
from concourse import library_config, mybir
from concourse.bass_utils import run_bass_kernel_spmd

TRACE = False
LAST_EXEC_NS = None
LAST_RESULTS = None

F32 = mybir.dt.float32
AX = mybir.AxisListType
ALU = mybir.AluOpType
ACT_F = mybir.ActivationFunctionType

B, S, H, D = 2, 1024, 16, 2048
N_CORES = 8
S_CHUNK = S // N_CORES        # 128 attn rows per core
BH = B * H                    # 32
DSL = D // N_CORES            # 256 Wd columns per core
NWT = D // 128                # 16 row-tiles of the Wd column slice
PG = 2                        # heads per attention piece
NP = BH // PG                 # 16 pieces
NCH = NP // 2                 # 8 ACT chain batches ([128, 4] each)
EPS = 1e-6
INV_D = 1.0 / D
Q_IN = 16                     # [128,*] DMA completion quantum
Q_CC = 32                     # [1,*] DMA completion quantum
THROTTLE = 3                  # max in-flight attention in-DMAs
N_HOIST = 10                  # reduces hoisted before the gate chain


def build_kernel(debug: bool = False, detect_races: bool = True):
    nc = bass.Bass(detect_race_conditions=detect_races)
    attn_in = nc.declare_dram_parameter("attn", [NP, 128, PG * S], F32, isOutput=False)
    xs_in = nc.declare_dram_parameter("xs", [128, B * D], F32, isOutput=False)
    wdc_in = nc.declare_dram_parameter("wdc", [128, NWT * DSL], F32, isOutput=False)
    bd_in = nc.declare_dram_parameter("bd", [1, D], F32, isOutput=False)
    gwb_in = nc.declare_dram_parameter("gwb", [1, 3], F32, isOutput=False)
    out_d = nc.declare_dram_parameter("out", [NP, 128, PG * S], F32, isOutput=True)
    cc_in = nc.dram_tensor("cc_in", [1, DSL], F32)
    cc_out = nc.dram_tensor("cc_out", [1, D], F32, addr_space="Shared")
    extras_dram = nc.dram_tensor("extras_dram", [1, 4], F32)

    ctx = ExitStack()
    with ctx:
        sb = lambda shape, name: ctx.enter_context(
            nc.sbuf_tensor(name, shape, F32))
        sem = lambda name: ctx.enter_context(nc.semaphore(name))

        tin = [sb([128, PG * S], f"tin{i}") for i in range(NP)]
        wd_sb = sb([128, NWT * DSL], "wd_sb")
        xt = sb([128, B * D], "xt")
        wbar_sb = sb([128, D], "wbar_sb")
        bd_sb = sb([1, D], "bd_sb")
        gwb_sb = sb([1, 3], "gwb_sb")
        cs_sb = sb([1, DSL], "cs_sb")
        cc_sb = sb([1, D], "cc_sb")
        dinit = sb([1, 1], "dinit")
        staging = sb([1, 4], "staging")
        extras_sb = sb([128, 4], "extras_sb")
        rs_all = sb([128, BH], "rs_all")
        rec_scr = sb([128, BH], "rec_scr")
        rec_all = sb([128, BH], "rec_all")
        sc_all = sb([128, BH], "sc_all")
        ghraw = sb([128, B], "ghraw")
        dcol = sb([128, B], "dcol")
        gden = sb([128, B], "gden")
        grscr = sb([128, B], "grscr")
        grec = sb([128, B], "grec")
        gt1 = sb([128, B], "gt1")
        gt1g = sb([128, B], "gt1g")
        gt2g = sb([128, B], "gt2g")
        etm = sb([128, B], "etm")
        cb = sb([128, B], "cb")
        ones_col = sb([128, 1], "ones_col")
        c001 = sb([128, 1], "c001")
        warm = sb([128, 512], "warm")
        colsum_ps = ctx.enter_context(nc.psum_tensor("colsum_ps", [1, DSL], F32))
        warm_ps = ctx.enter_context(nc.psum_tensor("warm_ps", [1, 512], F32))

        s_in = [sem(f"s_in{i}") for i in range(NP)]
        s_x = sem("s_x")
        s_wd = sem("s_wd")
        s_wd2 = sem("s_wd2")
        s_rs_d = sem("s_rs_d")
        s_rs_a = sem("s_rs_a")
        s_md = sem("s_md")
        s_rs_p = sem("s_rs_p")
        s_cci = sem("s_cci")
        s_cc = sem("s_cc")
        s_ws = sem("s_ws")
        s_exo = sem("s_exo")
        s_exb = sem("s_exb")
        s_ones = sem("s_ones")
        s_pe = sem("s_pe")
        s_colsum_sb = sem("s_colsum_sb")
        s_dinit = sem("s_dinit")
        s_staging = sem("s_staging")
        s_hemul = sem("s_hemul")
        s_ghr = sem("s_ghr")
        s_gden = sem("s_gden")
        s_grec = sem("s_grec")
        s_lin = sem("s_lin")
        s_et = sem("s_et")
        s_cb = sem("s_cb")
        s_rs = sem("s_rs")
        s_recact = sem("s_recact")
        s_sc = sem("s_sc")
        s_ma = sem("s_ma")
        s_vchain = sem("s_vchain")
        s_achain = sem("s_achain")
        s_sink = sem("s_sink")
        s_out = sem("s_out")

        with nc.Block() as block:

            # piece ownership: in-queues, reduce engines, out-queues
            IN_ACT = [0, 1, 2, 3, 4, 5]         # ACT HWDGE queue
            IN_SP = [10, 11, 6, 7, 8, 9]        # SP queue; 10/11 first (DVE needs them pre-he)
            IN_POOL = [12, 13, 14, 15]          # Pool queue, before cc_in
            RED_D = [12, 13, 14, 0, 1, 2, 10, 11, 8, 9]  # DVE, arrival order
            RED_A = [15, 3, 4, 5, 6, 7]                  # ACT accum reduces
            OUT_SP = [0, 2, 4, 6, 8, 10, 12]
            OUT_POOL = [1, 3, 5, 7, 9, 11, 13]
            OUT_ACT = [14, 15]                  # after ACT's last muls
            HWT = NWT // 2

            def chain_waits(eng, q):
                """Reduce-stream counts each chain batch q must wait for."""
                need = (2 * q, 2 * q + 1)
                order = {"d": RED_D, "a": RED_A}[eng]
                return max([order.index(p) + 1 for p in need if p in order],
                           default=0)

            @block.sync
            def _(sync):
                # wd first: it gates colsum -> AllGather -> gate.  bd/gwb
                # have no direct waiters - covered via queue FIFO by in6.
                sync.dma_start(
                    wd_sb[:, :HWT * DSL], wdc_in[:, :HWT * DSL]
                ).then_inc(s_wd, 16)
                sync.dma_start(
                    wd_sb[:, HWT * DSL:], wdc_in[:, HWT * DSL:]
                ).then_inc(s_wd2, 16)
                sync.dma_start(xt[:], xs_in[:]).then_inc(s_x, 16)
                sync.dma_start(bd_sb[:], bd_in[:]).then_inc(s_sink, 16)
                sync.dma_start(gwb_sb[:], gwb_in[:]).then_inc(s_sink, 16)
                for p in IN_SP:
                    sync.dma_start(
                        tin[p][:], attn_in[p]).then_inc(s_in[p], Q_IN)
                for p in OUT_SP:
                    sync.wait_ge(s_ma, p + 1)
                    sync.wait_ge(s_md, p + 1)
                    sync.dma_start(out_d[p], tin[p][:]).then_inc(s_sink, 16)

            @block.tensor
            def _(tensor):
                # warm the PE clock (pstate ramps with continuous busy time)
                tensor.wait_ge(s_ones, 1)
                for _ in range(8):
                    nc.tensor.matmul(
                        warm_ps[:], lhsT=ones_col[:], rhs=warm[:],
                        start=True, stop=True)
                tensor.wait_ge(s_wd, 16)
                for t in range(HWT):
                    nc.tensor.matmul(
                        colsum_ps[:],
                        lhsT=ones_col[:],
                        rhs=wd_sb[:, t * DSL:(t + 1) * DSL],
                        start=(t == 0), stop=False)
                tensor.wait_ge(s_wd2, 16)
                for t in range(HWT, NWT):
                    mm = nc.tensor.matmul(
                        colsum_ps[:],
                        lhsT=ones_col[:],
                        rhs=wd_sb[:, t * DSL:(t + 1) * DSL],
                        start=False, stop=(t == NWT - 1))
                mm.then_inc(s_pe, 1)

            @block.gpsimd
            def _(gpsimd):
                for p in IN_POOL:
                    gpsimd.dma_start(
                        tin[p][:], attn_in[p]).then_inc(s_in[p], Q_IN)
                gpsimd.wait_ge(s_colsum_sb, 1)
                gpsimd.dma_start(cc_in[:], cs_sb[:]).then_inc(s_cci, Q_CC)
                gpsimd.wait_ge(s_cci, Q_CC)
                gpsimd.collective_compute(
                    "AllGather",
                    ALU.bypass,
                    replica_groups=[list(range(N_CORES))],
                    ins=[cc_in[:]],
                    outs=[cc_out[:]],
                ).then_inc(s_cc, 1)
                gpsimd.wait_ge(s_cc, 1)
                gpsimd.dma_start(
                    wbar_sb[:], cc_out[:].broadcast_to((128, D))
                ).then_inc(s_ws, 16)
                gpsimd.wait_ge(s_staging, 1)
                gpsimd.dma_start(extras_dram[:], staging[:]).then_inc(s_exo, 16)
                gpsimd.wait_ge(s_exo, 16)
                gpsimd.dma_start(
                    extras_sb[:], extras_dram[:].broadcast_to((128, 4))
                ).then_inc(s_exb, 16)
                for p in OUT_POOL:
                    gpsimd.wait_ge(s_ma, p + 1)
                    gpsimd.wait_ge(s_md, p + 1)
                    gpsimd.dma_start(out_d[p], tin[p][:]).then_inc(s_out, 16)

            @block.vector
            def _(vector):
                vc = 0
                nc.vector.memset(warm[:], 1.0)
                nc.vector.memset(c001[:], 0.01)
                nc.vector.memset(ones_col[:], 1.0).then_inc(s_ones, 1)
                # row-sum reduces, ordered by expected piece arrival;
                # cs-copy / staging slot in between the early ones
                for i, p in enumerate(RED_D):
                    if i == 2:
                        # PE colsum lands about now; ship it to the AllGather
                        vector.wait_ge(s_pe, 1)
                        nc.vector.tensor_copy(
                            cs_sb[:], colsum_ps[:]).then_inc(s_colsum_sb, 1)
                    vector.wait_ge(s_in[p], Q_IN)
                    nc.vector.reduce_sum(
                        rs_all[:, p * PG:(p + 1) * PG],
                        tin[p].rearrange("p (g t) -> p g t", g=PG),
                        axis=AX.X).then_inc(s_rs_d, 1)
                # staging = [gW00, 0.1*gW01, -gb, mean(bd)+EPS]
                vector.wait_ge(s_in[10], Q_IN)  # bd+gwb (queue FIFO)
                vector.wait_ge(s_dinit, 1)
                nc.vector.tensor_copy(staging[:, 0:1], gwb_sb[:, 0:1])
                nc.vector.tensor_scalar(
                    out=staging[:, 1:2], in0=gwb_sb[:, 1:2],
                    scalar1=0.1, scalar2=None, op0=ALU.mult)
                nc.vector.tensor_scalar(
                    out=staging[:, 2:3], in0=gwb_sb[:, 2:3],
                    scalar1=-1.0, scalar2=None, op0=ALU.mult)
                nc.vector.tensor_copy(
                    staging[:, 3:4], dinit[:]).then_inc(s_staging, 1)
                # he multiplies (ACT accumulates them into ghraw)
                vector.wait_ge(s_ws, 16)
                vector.wait_ge(s_x, 16)
                for b in range(B):
                    nc.vector.tensor_mul(
                        xt[:, b * D:(b + 1) * D], xt[:, b * D:(b + 1) * D],
                        wbar_sb[:]).then_inc(s_hemul, 1)
                # gate chain; same-engine dependent pairs completion-synced
                vector.wait_ge(s_ghr, B)
                vector.wait_ge(s_exb, 16)
                nc.vector.tensor_scalar(
                    out=dcol[:], in0=ghraw[:],
                    scalar1=INV_D, scalar2=extras_sb[:, 3:4],
                    op0=ALU.mult, op1=ALU.add).then_inc(s_vchain, 1)
                vc += 1; vector.wait_ge(s_vchain, vc)
                nc.vector.tensor_mul(gden[:], dcol[:], dcol[:]).then_inc(
                    s_gden, 1)
                nc.vector.tensor_scalar(
                    out=gt1[:], in0=dcol[:], scalar1=extras_sb[:, 0:1],
                    scalar2=extras_sb[:, 1:2], op0=ALU.mult, op1=ALU.add
                ).then_inc(s_vchain, 1)
                vc += 1; vector.wait_ge(s_vchain, vc)
                vector.wait_ge(s_grec, 1)
                nc.vector.tensor_mul(gt1g[:], gt1[:], grec[:]).then_inc(s_lin, 1)
                # cb = EPS * (1 + exp(-(lin+gb))) : the whole gate effect
                vector.wait_ge(s_et, 1)
                nc.vector.tensor_scalar(
                    out=cb[:], in0=etm[:], scalar1=EPS, scalar2=EPS,
                    op0=ALU.mult, op1=ALU.add).then_inc(s_cb, 1)
                # per batch: bounce sc (cross-engine scalar port), scale g=0
                for q in range(NCH + 1):
                    if q < NCH:
                        vector.wait_ge(s_recact, q + 1)
                        nc.vector.tensor_copy(
                            sc_all[:, q * 4:(q + 1) * 4],
                            rec_all[:, q * 4:(q + 1) * 4]).then_inc(s_sc, 1)
                        vector.wait_ge(s_sc, q + 1)
                    if q > 0:
                        for pp in range(2):
                            p = 2 * (q - 1) + pp
                            col = p * PG
                            nc.vector.tensor_scalar(
                                out=tin[p][:, 0:S], in0=tin[p][:, 0:S],
                                scalar1=sc_all[:, col:col + 1], scalar2=None,
                                op0=ALU.mult).then_inc(s_md, 1)

            @block.scalar
            def _(scalar):
                ac = 0
                for p in IN_ACT:
                    scalar.dma_start(
                        tin[p][:], attn_in[p]).then_inc(s_in[p], Q_IN)
                scalar.wait_ge(s_in[10], Q_IN)  # bd landed (queue FIFO)
                nc.scalar.activation(
                    bd_sb[:], bd_sb[:], ACT_F.Copy,
                    bias=EPS * INV_D, scale=INV_D, accum_out=dinit[:],
                ).then_inc(s_dinit, 1)
                # accum-reduces for the ACT-owned pieces
                for p in RED_A:
                    scalar.wait_ge(s_in[p], Q_IN)
                    for g in range(PG):
                        mi = nc.scalar.activation(
                            tin[p][:, g * S:(g + 1) * S],
                            tin[p][:, g * S:(g + 1) * S], ACT_F.Copy,
                            bias=0.0, scale=1.0,
                            accum_out=rs_all[:, p * PG + g:p * PG + g + 1])
                    mi.then_inc(s_rs_a, 1)
                # he accumulation: ghraw[:, b] = sum_d xt[:, b*D:(b+1)*D]
                for b in range(B):
                    scalar.wait_ge(s_hemul, b + 1)
                    nc.scalar.activation(
                        xt[:, b * D:(b + 1) * D], xt[:, b * D:(b + 1) * D],
                        ACT_F.Copy, bias=0.0, scale=1.0,
                        accum_out=ghraw[:, b:b + 1]).then_inc(s_ghr, 1)
                # grec = 1/(dcol^2 + 0.01) = exp(-ln(gden + 0.01))
                scalar.wait_ge(s_gden, 1)
                nc.scalar.activation(
                    grscr[:], gden[:], ACT_F.Ln,
                    bias=c001[:], scale=1.0).then_inc(s_achain, 1)
                ac += 1; scalar.wait_ge(s_achain, ac)
                nc.scalar.activation(
                    grec[:], grscr[:], ACT_F.Exp,
                    bias=0.0, scale=-1.0).then_inc(s_grec, 1)
                # etm = exp(-(lin + gb))   (bias AP holds -gb)
                scalar.wait_ge(s_lin, 1)
                nc.scalar.activation(
                    etm[:], gt1g[:], ACT_F.Exp,
                    bias=extras_sb[:, 2:3], scale=-1.0).then_inc(s_et, 1)
                # per chain batch q (pieces 2q, 2q+1; 4 bh columns):
                #   rec = exp(-ln(rs + cb)) ; scale g=1 rows in place
                scalar.wait_ge(s_cb, 1)
                for q in range(NCH + 1):
                    if q < NCH:
                        b = q // (NCH // B)
                        cols = slice(q * 4, (q + 1) * 4)
                        for eng, sem_h in (("d", s_rs_d), ("a", s_rs_a)):
                            n = chain_waits(eng, q)
                            if n:
                                scalar.wait_ge(sem_h, n)
                        nc.scalar.activation(
                            rec_scr[:, cols], rs_all[:, cols], ACT_F.Ln,
                            bias=cb[:, b:b + 1], scale=1.0).then_inc(s_achain, 1)
                        ac += 1; scalar.wait_ge(s_achain, ac)
                        nc.scalar.activation(
                            rec_all[:, cols], rec_scr[:, cols], ACT_F.Exp,
                            bias=0.0, scale=-1.0).then_inc(s_recact, 1)
                    if q > 0:
                        for pp in range(2):
                            p = 2 * (q - 1) + pp
                            col = p * PG + 1
                            if pp == 0:
                                scalar.wait_ge(s_sc, q)
                            nc.scalar.activation(
                                tin[p][:, S:2 * S],
                                tin[p][:, S:2 * S], ACT_F.Copy,
                                bias=0.0, scale=sc_all[:, col:col + 1]
                            ).then_inc(s_ma, 1)
                # last two outs ride the ACT queue once its muls are done
                scalar.wait_ge(s_ma, NP)  # own g=1 muls complete
                for p in OUT_ACT:
                    scalar.wait_ge(s_md, p + 1)
                    scalar.dma_start(out_d[p], tin[p][:]).then_inc(s_sink, 16)
    return nc


_NC_CACHE = {}


def _get_nc():
    if "nc" not in _NC_CACHE:
        _NC_CACHE["nc"] = build_kernel()
    return _NC_CACHE["nc"]


def kernel(x, attention_weights, Wd, bd, Wsup, bsup, Wsub, bsub, gW, gb):
    """Full inputs in, full output out; shards internally across 8 cores."""
    global LAST_EXEC_NS, LAST_RESULTS
    x = np.ascontiguousarray(x, dtype=np.float32)
    attention_weights = np.ascontiguousarray(attention_weights, dtype=np.float32)
    Wd = np.ascontiguousarray(Wd, dtype=np.float32)
    bd_r = np.asarray(bd, dtype=np.float32).reshape(1, D)
    gwb = np.array([[np.float32(gW[0, 0]), np.float32(gW[0, 1]),
                     np.float32(gb[0])]], dtype=np.float32)

    nc = _get_nc()

    in_maps = []
    for k in range(N_CORES):
        sk = k * S_CHUNK
        ck = k * DSL
        # attn: [NP, 128, PG*S] with tin[p][s, g*S+t] = attn[bh=p*PG+g, s, t]
        attn_relay = np.ascontiguousarray(
            attention_weights[:, :, sk:sk + S_CHUNK, :]
            .reshape(NP, PG, S_CHUNK, S)
            .transpose(0, 2, 1, 3)
            .reshape(NP, 128, PG * S))
        # x: [128, B*D] with xt[s, b*D+d] = x[b, sk+s, d]
        xs_relay = np.ascontiguousarray(
            x[:, sk:sk + S_CHUNK, :].transpose(1, 0, 2).reshape(128, B * D))
        # wd: [128, NWT*DSL] with wd_sb[p, t*DSL+c] = Wd[t*128+p, ck+c]
        wd_relay = np.ascontiguousarray(
            Wd[:, ck:ck + DSL].reshape(NWT, 128, DSL)
            .transpose(1, 0, 2).reshape(128, NWT * DSL))
        in_maps.append({
            "attn": attn_relay,
            "xs": xs_relay,
            "wdc": wd_relay,
            "bd": bd_r,
            "gwb": gwb,
        })

    res = run_bass_kernel_spmd(nc, in_maps, list(range(N_CORES)), trace=TRACE)
    LAST_EXEC_NS = res.exec_time_ns
    LAST_RESULTS = res
    out = np.empty((B, H, S, S), dtype=np.float32)
    for k in range(N_CORES):
        sk = k * S_CHUNK
        out[:, :, sk:sk + S_CHUNK, :] = (
            res.results[k]["out"]
            .reshape(NP, S_CHUNK, PG, S)
            .transpose(0, 2, 1, 3)
            .reshape(B, H, S_CHUNK, S))
    return out
